# revision 17
# baseline (speedup 1.0000x reference)
"""MoE (8 experts, top-2) expert-parallel kernel for 8 TRN2 NeuronCores.

Contract: kernel(**inputs) takes the FULL unsharded inputs and returns the
FULL output [2, 2048, 1024] fp32.

Strategy (expert parallelism, host-side dispatch/combine):
  - Router (x @ Wr + biases, top-2, softmax) is computed on host — it is
    0.03% of the FLOPs; the dispatch it implies IS the input sharding.
  - Core e receives exactly the tokens routed to expert e (gathered,
    transposed to [D, C], zero-padded to capacity C) plus W1[e]/b1[e]/W2[e].
  - On-device per core: y^T = W2[e]^T-tiles @ gelu(W1[e]-tiles^T @ x^T + b1)
    with fp32r matmuls (full-rate fp32 on the PE array), weights streamed
    from HBM exactly once, h accumulated H-chunk-wise through PSUM, y
    accumulated in SBUF.
  - Host combine: out[tokens_e] += gate_e * (y_e + b2[e])  (weighted
    "all-to-all back" equivalent), summing the two expert contributions
    per token.

Capacity C adapts to the observed max expert load (rounded up to 128,
min 256); distinct capacities compile distinct NEFFs (cached in-process).
Any tokens beyond a compiled capacity would be handled exactly on host —
with C = rounded-up max load this path never triggers.
"""

import numpy as np

import concourse.bass as bass  # noqa: F401  (bass types used via bacc/tile)
import concourse.mybir as mybir
import concourse.tile as tile
from concourse import bacc
from concourse.bass_utils import run_bass_kernel_spmd

E = 8
TOPK = 2
D = 1024
H = 4096
P = 128
KD = D // P   # 8  k-tiles over D
HT = H // P   # 32 h-tiles over H
DT = D // P   # 8  d-tiles over D
G = 4         # h-tiles per weight-resident chunk

_nc_cache: dict[tuple, object] = {}


def _make_blocks(c: int) -> tuple:
    """Split capacity c (multiple of 128, >=256) into token blocks.

    Every block must be >=256 (fp32r full-rate moving dim) and <=512
    (PSUM bank / fp32 moving-operand limit).
    """
    blocks = []
    rem = c
    while rem > 0:
        if rem <= 512:
            blocks.append(rem)
            break
        if rem - 512 >= 256:
            blocks.append(512)
            rem -= 512
        else:  # rem in (512, 768): split as (rem-256, 256)
            blocks.append(rem - 256)
            blocks.append(256)
            break
    return tuple(blocks)


def _build(blocks: tuple):
    """Build + compile the single-core expert-MLP program for one capacity."""
    C = sum(blocks)
    f32 = mybir.dt.float32
    f32r = mybir.dt.float32r
    AF = mybir.ActivationFunctionType

    nc = bacc.Bacc(None, target_bir_lowering=False, debug=False)
    xt = nc.dram_tensor("xt", [D, C], f32r, kind="ExternalInput")
    w1 = nc.dram_tensor("w1", [HT, P, KD, P], f32r, kind="ExternalInput")
    w2 = nc.dram_tensor("w2", [HT, P, D], f32r, kind="ExternalInput")
    b1v = nc.dram_tensor("b1v", [H], f32, kind="ExternalInput")
    yt = nc.dram_tensor("yt", [D, C], f32, kind="ExternalOutput")

    # Blocks smallest-first: the first accumulation group only needs the
    # smallest xt slice + first weight tile, shrinking the PE head stall.
    blocks = tuple(sorted(blocks))
    offs = [sum(blocks[:i]) for i in range(len(blocks))]
    NB = len(blocks)
    NCHUNK = HT // G

    with tile.TileContext(nc) as tc:
        with (
            tc.tile_pool(name="big", bufs=1) as big,
            tc.tile_pool(name="w1p", bufs=3) as w1p,
            tc.tile_pool(name="w2p", bufs=3) as w2p,
            tc.tile_pool(name="hp", bufs=2) as hp,
            tc.tile_pool(name="php", bufs=2, space="PSUM") as php,
            tc.tile_pool(name="pyp", bufs=4, space="PSUM") as pyp,
        ):
            b1_sb = big.tile([P, HT], f32)
            # Per-block xt tiles (one DMA each), emitted in first-use order:
            # block 0 first, chunk-0 weights next, remaining blocks after.
            xt_r = xt.rearrange("(k p) c -> p k c", p=P)
            xt_t = [None] * NB

            def load_xt(b):
                t = big.tile([P, KD, blocks[b]], f32r, tag=f"xt_{b}",
                             name=f"xt_{b}")
                nc.sync.dma_start(t[:], xt_r[:, :, offs[b]:offs[b] + blocks[b]])
                xt_t[b] = t

            load_xt(0)
            y_t = [[big.tile([P, blocks[b]], f32, tag=f"y_{dd}_{b}",
                             name=f"y_{dd}_{b}")
                    for b in range(NB)] for dd in range(DT)]
            yt_r = yt.rearrange("(d p) c -> p d c", p=P)

            for chunk in range(NCHUNK):
                w1_t, w2_t = [], []
                for ii in range(G):
                    i = chunk * G + ii
                    w1t = w1p.tile([P, KD, P], f32r, tag=f"w1_{ii}")
                    nc.sync.dma_start(w1t[:], w1[i])
                    if chunk == 0 and ii == 0:
                        # b1 first used by the first gelu, well after MM start
                        nc.sync.dma_start(
                            b1_sb[:], b1v.rearrange("(j p) -> p j", p=P)
                        )
                    w2t = w2p.tile([P, D], f32r, tag=f"w2_{ii}")
                    nc.sync.dma_start(w2t[:], w2[i])
                    w1_t.append(w1t)
                    w2_t.append(w2t)
                if chunk == 0:
                    for b in range(1, NB):
                        load_xt(b)
                for b, nb in enumerate(blocks):
                    h_t = []
                    for ii in range(G):
                        i = chunk * G + ii
                        ph = php.tile([P, nb], f32, tag="ph")
                        for k in range(KD):
                            nc.tensor.matmul(
                                ph[:],
                                w1_t[ii][:, k, :],
                                xt_t[b][:, k, :],
                                start=(k == 0),
                                stop=(k == KD - 1),
                            )
                        ht = hp.tile([P, nb], f32r, tag=f"h_{ii}")
                        nc.scalar.activation(
                            ht[:], ph[:], AF.Gelu, bias=b1_sb[:, i:i + 1]
                        )
                        h_t.append(ht)
                    for dd in range(DT):
                        py = pyp.tile([P, nb], f32, tag="py")
                        for ii in range(G):
                            nc.tensor.matmul(
                                py[:],
                                w2_t[ii][:, dd * P:(dd + 1) * P],
                                h_t[ii][:],
                                start=(ii == 0),
                                stop=(ii == G - 1),
                            )
                        dst = y_t[dd][b]
                        if chunk == 0:
                            nc.vector.tensor_copy(dst[:], py[:])
                        else:
                            nc.vector.tensor_add(dst[:], dst[:], py[:])
                        if chunk == NCHUNK - 1:
                            # Region final — stream it out now.
                            nc.sync.dma_start(
                                yt_r[:, dd, offs[b]:offs[b] + nb], dst[:]
                            )
    nc.compile()
    return nc


def _get_nc(blocks: tuple):
    nc = _nc_cache.get(blocks)
    if nc is None:
        nc = _build(blocks)
        _nc_cache[blocks] = nc
    return nc


class _Runner:
    """Cached SPMD executor for one compiled program.

    run_bass_kernel_spmd re-traces, re-jits, and re-uploads all inputs
    (incl. 270 MB of expert weights) through the axon tunnel on every
    call. This runner jits once and keeps the weights device-resident
    across calls (re-uploading only when their content hash changes), so
    steady-state calls ship just the routed tokens.
    """

    def __init__(self, nc):
        import jax
        from concourse import bass2jax

        bass2jax.install_neuronx_cc_hook()
        self._bass2jax = bass2jax
        self.nc = nc
        assert nc.dbg_addr is None
        pid_name = (
            nc.partition_id_tensor.name if nc.partition_id_tensor else None
        )
        import concourse.mybir as mb

        in_names, out_names, out_avals, zero_shapes = [], [], [], []
        for alloc in nc.m.functions[0].allocations:
            if not isinstance(alloc, mb.MemoryLocationSet):
                continue
            name = alloc.memorylocations[0].name
            if alloc.kind == "ExternalInput":
                if name != pid_name:
                    in_names.append(name)
            elif alloc.kind == "ExternalOutput":
                shape = tuple(alloc.tensor_shape)
                dtype = mb.dt.np(alloc.dtype)
                out_names.append(name)
                out_avals.append(jax.core.ShapedArray(shape, dtype))
                zero_shapes.append((shape, dtype))
        self.in_names = list(in_names)
        self.out_names = out_names
        self.out_avals = out_avals
        self.zero_shapes = zero_shapes
        bind_names = tuple(
            in_names + out_names + ([pid_name] if pid_name else [])
        )

        def _body(*args):
            operands = list(args)
            if pid_name is not None:
                operands.append(bass2jax.partition_id_tensor())
            outs = bass2jax._bass_exec_p.bind(
                *operands,
                out_avals=tuple(out_avals),
                in_names=bind_names,
                out_names=tuple(out_names),
                lowering_input_output_aliases=(),
                sim_require_finite=True,
                sim_require_nnan=True,
                nc=nc,
            )
            return tuple(outs)

        devices = jax.devices()[:E]
        self.mesh = bass2jax.Mesh(np.asarray(devices), ("core",))
        self.pspec = bass2jax.PartitionSpec("core")
        n_ops = len(in_names) + len(out_names)
        self.jitted = jax.jit(
            bass2jax.shard_map(
                _body,
                mesh=self.mesh,
                in_specs=(self.pspec,) * n_ops,
                out_specs=(self.pspec,) * len(out_names),
                check_rep=False,
            ),
            keep_unused=True,
        )
        self.sharding = jax.sharding.NamedSharding(self.mesh, self.pspec)
        self._static_cache = {}  # name -> (digest, device_array)
        self._zeros = None

    @staticmethod
    def _digest(arrs):
        import hashlib

        h = hashlib.blake2b(digest_size=16)
        for a in arrs:
            h.update(np.ascontiguousarray(a).data)
        return h.digest()

    def _put(self, name, per_core, static):
        import jax

        glob = np.concatenate([np.asarray(a) for a in per_core], axis=0)
        if not static:
            return jax.device_put(glob, self.sharding)
        dig = self._digest(per_core)
        hit = self._static_cache.get(name)
        if hit is not None and hit[0] == dig:
            return hit[1]
        arr = jax.device_put(glob, self.sharding)
        self._static_cache[name] = (dig, arr)
        return arr

    def run(self, in_maps, static_names):
        import jax

        ops = [
            self._put(nm, [m[nm] for m in in_maps], nm in static_names)
            for nm in self.in_names
        ]
        if self._zeros is None:
            self._zeros = [
                jax.device_put(
                    np.zeros((E * s[0], *s[1:]), dt), self.sharding
                )
                for s, dt in self.zero_shapes
            ]
        outs = self.jitted(*ops, *self._zeros)
        results = []
        for c in range(E):
            results.append({
                nm: np.asarray(outs[i]).reshape(E, *self.out_avals[i].shape)[c]
                for i, nm in enumerate(self.out_names)
            })
        return results


_runner_cache: dict[tuple, _Runner] = {}
_STATIC_NAMES = frozenset({"w1", "w2", "b1v"})


def _run(blocks, in_maps):
    """Execute on the 8 cores; cached fast path with spmd fallback."""
    nc = _get_nc(blocks)
    try:
        runner = _runner_cache.get(blocks)
        if runner is None:
            runner = _Runner(nc)
            _runner_cache[blocks] = runner
        return runner.run(in_maps, _STATIC_NAMES)
    except Exception:
        return run_bass_kernel_spmd(
            nc, in_maps, core_ids=list(range(E))
        ).results


def _route(x, Wr, br, gate_bias):
    """Top-2 routing. Returns (token_idx per expert, gate weight per expert)."""
    logits = x @ Wr + br + gate_bias
    top2 = np.argpartition(-logits, TOPK - 1, axis=1)[:, :TOPK]
    tv = np.take_along_axis(logits, top2, axis=1)
    tv = tv - tv.max(axis=1, keepdims=True)
    pe = np.exp(tv)
    pe /= pe.sum(axis=1, keepdims=True)
    idx_e, gate_e = [], []
    for e in range(E):
        rows, cols = np.nonzero(top2 == e)  # each token at most once per expert
        idx_e.append(rows.astype(np.int64))
        gate_e.append(pe[rows, cols].astype(np.float32))
    return idx_e, gate_e


def kernel(hidden_states, Wr, br, gate_bias, W1, b1, W2, b2):
    B, S, Din = hidden_states.shape
    x = np.ascontiguousarray(hidden_states.reshape(B * S, Din), dtype=np.float32)
    Wr = np.asarray(Wr, np.float32)
    br = np.asarray(br, np.float32)
    gate_bias = np.asarray(gate_bias, np.float32)
    W1 = np.asarray(W1, np.float32)
    b1 = np.asarray(b1, np.float32)
    W2 = np.asarray(W2, np.float32)
    b2 = np.asarray(b2, np.float32)

    idx_e, gate_e = _route(x, Wr, br, gate_bias)
    max_cnt = max(len(ix) for ix in idx_e)
    C = max(256, -(-max_cnt // P) * P)
    blocks = _make_blocks(C)

    in_maps = []
    for e in range(E):
        ix = idx_e[e][:C]  # overflow beyond C handled exactly on host below
        xt = np.zeros((D, C), np.float32)
        xt[:, :len(ix)] = x[ix].T
        in_maps.append({
            "xt": xt,
            "w1": np.ascontiguousarray(
                W1[e].reshape(KD, P, HT, P).transpose(2, 1, 0, 3)
            ),
            "w2": np.ascontiguousarray(W2[e].reshape(HT, P, D)),
            "b1v": np.ascontiguousarray(b1[e]),
        })

    results = _run(blocks, in_maps)

    out = np.zeros((B * S, D), np.float32)
    for e in range(E):
        ix = idx_e[e]
        g = gate_e[e]
        n = min(len(ix), C)
        y = results[e]["yt"][:, :n].T + b2[e][None, :]
        out[ix[:n]] += g[:n, None] * y
        if len(ix) > C:  # exact host fallback; unreachable with adaptive C
            xo = x[ix[C:]].astype(np.float64)
            h = xo @ W1[e].astype(np.float64) + b1[e]
            from scipy.special import erf
            h = 0.5 * h * (1.0 + erf(h / np.sqrt(2.0)))
            yo = h @ W2[e].astype(np.float64) + b2[e]
            out[ix[C:]] += (g[C:, None] * yo).astype(np.float32)

    return out.reshape(B, S, D).astype(np.float32)
